# revision 44
# baseline (speedup 1.0000x reference)
"""Bilinear-sampling + global average pooling on 8 Trainium2 NeuronCores.

Math: out[b,c] = mean_{h,w} bilinear(data[b,c], grid + ts*offset[b])
The gather indices/weights depend only on (b,h,w), never on c, so the whole
op is a linear functional over spatial positions applied per channel:

    out[b,c] = (1/S) * sum_s A[b,s] * data[b,c,s]      (s = flattened H*W)

where A is the scatter-accumulation of the four bilinear corner weights of
every sample point.  A is computed on host from `offset` (131K elements,
0.1% of `data`); the device kernel does the memory-bound weighted reduction
over the `data` tensor.

Device kernel (per core, 4 batches), optimized for HBM bandwidth:
  - data and A are shipped as fp16 (halves HBM traffic; rel err ~4e-4,
    well under the 2e-2 gate).  All reductions accumulate in fp32.
  - input DMAs alternate between the two hardware DGE queues
    (qSyncDynamicHW via nc.sync, qScalarDynamicHW via nc.scalar) so both
    queues stream concurrently.
  - per [128, 4096] slab: one DVE tensor_tensor multiply, then the
    free-dim reduction on either ACT (activation Copy + accum_out) or DVE
    (tensor_reduce), statically load-balanced across the two engines.
    (tensor_tensor_reduce and fp32 matmul wedge this container's HW.)
  - GPSIMD partition_broadcast replicates A[b] across 128 partitions,
    overlapped with the data DMAs.
  - single [128, 8] fp32 output DMA; host applies the exact 1/S = 2^-12
    scale and reassembles [B, C].
  - Sharding: data-parallel over batch, 4 batches per core.
"""

import os
import sys

import numpy as np

for _p in ("/opt/trn_rl_repo", "/root/.axon_site/_ro/trn_rl_repo"):
    if os.path.isdir(_p) and _p not in sys.path:
        sys.path.append(_p)

import concourse.bacc as bacc
import concourse.mybir as mybir
import concourse.tile as tile
from concourse.bass_utils import run_bass_kernel_spmd

N_CORES = 8
B, C, H, W = 32, 256, 64, 64
S = H * W            # 4096 spatial positions
NB = B // N_CORES    # 4 batches per core
NCH = C // 128       # 2 channel halves of 128 partitions
NSLAB = NB * NCH     # 8 [128, S] data slabs per core

# TT emission order = expected data-arrival order (batch 3 on ring 0
# lands before batch 2), and which slabs' reductions run on ACT (True)
# vs DVE (False; emitted after all TTs since DVE is in-order).
SLAB_ORDER = (0, 1, 2, 3, 6, 7, 4, 5)
ACT_REDUCE = (True, True, True, True, True, False, True, True)

_CACHE = {}
LAST_RESULTS = None  # BassKernelResults of the most recent run (for test.py)


def _build_nc():
    nc = bacc.Bacc("TRN2", target_bir_lowering=False, debug=False,
                   num_devices=N_CORES, num_swdge_queues=2)
    f16 = mybir.dt.float16
    f32 = mybir.dt.float32
    i16 = mybir.dt.int16
    # 4 "pair" tensors, one per batch, its two channel-half slabs side by
    # side.  Each pair is fetched as two full-128-partition half-DMAs (one
    # per HW queue; partition-split DMAs crater queue throughput) so batch
    # b's data completes right as its A broadcast does.
    x = nc.dram_tensor("x", [NB, 128, NCH * S], f16, kind="ExternalInput")
    aw = nc.dram_tensor("aw", [1, NB * S], f16, kind="ExternalInput")
    y = nc.dram_tensor("y", [128, NSLAB], f32, kind="ExternalOutput")
    xt, at, yt = x.ap(), aw.ap(), y.ap()
    mult = mybir.AluOpType.mult
    add = mybir.AluOpType.add

    with tile.TileContext(nc) as tc:
        with (
            tc.tile_pool(name="arow", bufs=1) as arowp,
            tc.tile_pool(name="abc", bufs=1) as abcp,
            tc.tile_pool(name="data", bufs=1) as datap,
            tc.tile_pool(name="junk", bufs=1) as junkp,
            tc.tile_pool(name="col", bufs=1) as colp,
        ):
            # A replicated across partitions by DRAM->SBUF DMAs with a
            # stride-0 source AP (rows re-read per partition; row-buffer
            # friendly), on gpsimd's software DGE queue -- keeps the two HW
            # queues free for data and gpsimd's ALU idle (its compute ops
            # share SBUF ports with DVE and throttle it 3.5x).  Two batches
            # per DMA: the A rows are contiguous in DRAM, so each transfer
            # gets 16KB packets (~140GB/s vs ~110 at 8KB) while batch pairs
            # still arrive staggered.
            # Both paired A-broadcasts ride SWDGE ring 1 via dma_gather with
            # all-zero indices (128 copies of the 16KB row; the gather path
            # moves ~264GB/s, far above the plain queues).  The zero index
            # tile is built with a gpsimd memset -- a DMA of 128 tiny 16B
            # lines would clog a queue head for ~15us.
            idz_sb = arowp.tile([128, 8], i16)
            nc.gpsimd.memset(idz_sb[:], 0)
            abcp2 = []
            for p in range(NB // 2):
                t = abcp.tile([128, 2 * S], f16, name=f"abc{p}")
                nc.gpsimd.dma_gather(
                    t[:].rearrange("p (a b) -> p a b", a=1),
                    at[0:1, 2 * p * S : 2 * (p + 1) * S], idz_sb[:],
                    128, 128, 2 * S, elem_step=NB * S, queue_num=1)
                abcp2.append(t)
            abc = [abcp2[b // 2][:, (b % 2) * S : (b % 2 + 1) * S]
                   for b in range(NB)]

            # batches 0-2: halves split across the two HW queues; batch 3's
            # halves ride SWDGE ring 0 (otherwise idle), so all four queues
            # (sync, scalar, ring0, ring1) close together around 34us.
            dp = []
            for j in range(NB):
                d = datap.tile([128, NCH * S], f16, name=f"dp{j}")
                if j < NB - 1:
                    nc.sync.dma_start(d[:, 0:S], xt[j][:, 0:S])
                    nc.scalar.dma_start(d[:, S : 2 * S], xt[j][:, S : 2 * S])
                else:
                    nc.gpsimd.dma_start(d[:, 0:S], xt[j][:, 0:S])
                    nc.gpsimd.dma_start(d[:, S : 2 * S], xt[j][:, S : 2 * S])
                dp.append(d)

            prods = [junkp.tile([128, S], f16, name=f"prod{j}")
                     for j in range(4)]
            ajunk = junkp.tile([128, S], f16)  # ACT's mandatory full out
            segs = colp.tile([128, 64], f32)
            cols = colp.tile([128, NSLAB], f32)
            # Emission in data-ARRIVAL order (batch 3 rides the early-
            # finishing ring 0, so its slabs land before batch 2's).  The
            # last-arriving slab's reduce runs on DVE after all TTs so the
            # final two reductions run on ACT and DVE in parallel.
            dve_red = []
            for k, i in enumerate(SLAB_ORDER):
                b, l = i // NCH, i % NCH
                prod = prods[k % 4]
                nc.vector.tensor_tensor(
                    out=prod[:], in0=dp[b][:, l * S : (l + 1) * S],
                    in1=abc[b], op=mult)
                if ACT_REDUCE[i]:
                    nc.scalar.activation(
                        ajunk[:], prod[:],
                        mybir.ActivationFunctionType.Copy,
                        accum_out=cols[:, i : i + 1])
                else:
                    dve_red.append((i, prod))
            for i, prod in dve_red:
                nc.vector.tensor_reduce(
                    cols[:, i : i + 1], prod[:],
                    axis=mybir.AxisListType.X, op=add)
            nc.sync.dma_start(yt[:, :], cols[:])

    nc.compile()
    return nc


def _weight_field(offset, trans_std):
    """A[b,s]: accumulated bilinear weights per source pixel (UNnormalized:
    the 1/(H*W) mean factor is applied on host after the device reduction).

    Mirrors the reference coordinate math in float32.
    """
    offset = np.asarray(offset, np.float32)
    ts = np.float32(min(max(float(trans_std), 0.001), 0.01))
    ii = np.arange(H, dtype=np.float32)[None, :, None]
    jj = np.arange(W, dtype=np.float32)[None, None, :]
    y = np.clip(ii + ts * offset[:, 0] * np.float32(H),
                np.float32(0.0), np.float32(H - 1))
    x = np.clip(jj + ts * offset[:, 1] * np.float32(W),
                np.float32(0.0), np.float32(W - 1))
    y0 = np.clip(np.floor(y).astype(np.int32), 0, H - 2)
    x0 = np.clip(np.floor(x).astype(np.int32), 0, W - 2)
    wy = (y - y0.astype(np.float32)).astype(np.float64)
    wx = (x - x0.astype(np.float32)).astype(np.float64)

    base = np.arange(offset.shape[0], dtype=np.int64)[:, None, None] * S
    i00 = (y0.astype(np.int64) * W + x0 + base).ravel()
    i01 = i00 + 1
    i10 = i00 + W
    i11 = i10 + 1
    n = offset.shape[0] * S
    acc = (
        np.bincount(i00, ((1 - wy) * (1 - wx)).ravel(), minlength=n)
        + np.bincount(i01, ((1 - wy) * wx).ravel(), minlength=n)
        + np.bincount(i10, (wy * (1 - wx)).ravel(), minlength=n)
        + np.bincount(i11, (wy * wx).ravel(), minlength=n)
    )
    return acc.astype(np.float32).reshape(offset.shape[0], S)


def kernel(data, offset, trans_std):
    global LAST_RESULTS
    data = np.asarray(data, np.float32)
    offset = np.asarray(offset, np.float32)
    ts = float(np.asarray(trans_std).reshape(()))

    aw = _weight_field(offset, ts)  # [B, S] f32, unnormalized

    if "nc" not in _CACHE:
        _CACHE["nc"] = _build_nc()
    nc = _CACHE["nc"]

    # x shard layout: NB pairs of [128, 2*S] fp16; pair j = batch j's two
    # channel-half slabs side by side.
    xs = data.reshape(B, NCH, 128, S)
    in_maps = []
    for i in range(N_CORES):
        xi = (xs[i * NB : (i + 1) * NB].transpose(0, 2, 1, 3)
              .reshape(NB, 128, NCH * S).astype(np.float16))
        ai = aw[i * NB : (i + 1) * NB].reshape(1, NB * S).astype(np.float16)
        in_maps.append({"x": np.ascontiguousarray(xi),
                        "aw": np.ascontiguousarray(ai)})

    res = run_bass_kernel_spmd(nc, in_maps, core_ids=list(range(N_CORES)))
    LAST_RESULTS = res
    # y[p, b*NCH+ch] -> out[b, ch*128+p];  1/S == 2^-12 is exact in fp32.
    out = np.concatenate(
        [
            res.results[i]["y"].reshape(128, NB, NCH)
            .transpose(1, 2, 0).reshape(NB, C)
            for i in range(N_CORES)
        ],
        axis=0,
    )
    return np.ascontiguousarray((out * np.float32(1.0 / S)).astype(np.float32))


# revision 45
# speedup vs baseline: 1.1718x; 1.1718x over previous
"""Bilinear-sampling + global average pooling on 8 Trainium2 NeuronCores.

Math: out[b,c] = mean_{h,w} bilinear(data[b,c], grid + ts*offset[b])
The gather indices/weights depend only on (b,h,w), never on c, so the whole
op is a linear functional over spatial positions applied per channel:

    out[b,c] = (1/S) * sum_s A[b,s] * data[b,c,s]      (s = flattened H*W)

where A is the scatter-accumulation of the four bilinear corner weights of
every sample point.  A is computed on host from `offset` (131K elements,
0.1% of `data`); the device kernel does the memory-bound weighted reduction
over the `data` tensor.

Device kernel (per core, 4 batches), optimized for HBM bandwidth:
  - data and A are shipped as fp16 (halves HBM traffic; rel err ~4e-4,
    well under the 2e-2 gate).  All reductions accumulate in fp32.
  - input DMAs alternate between the two hardware DGE queues
    (qSyncDynamicHW via nc.sync, qScalarDynamicHW via nc.scalar) so both
    queues stream concurrently.
  - per [128, 4096] slab: one DVE tensor_tensor multiply, then the
    free-dim reduction on either ACT (activation Copy + accum_out) or DVE
    (tensor_reduce), statically load-balanced across the two engines.
    (tensor_tensor_reduce and fp32 matmul wedge this container's HW.)
  - GPSIMD partition_broadcast replicates A[b] across 128 partitions,
    overlapped with the data DMAs.
  - single [128, 8] fp32 output DMA; host applies the exact 1/S = 2^-12
    scale and reassembles [B, C].
  - Sharding: data-parallel over batch, 4 batches per core.
"""

import os
import sys

import numpy as np

for _p in ("/opt/trn_rl_repo", "/root/.axon_site/_ro/trn_rl_repo"):
    if os.path.isdir(_p) and _p not in sys.path:
        sys.path.append(_p)

import concourse.bacc as bacc
import concourse.mybir as mybir
import concourse.tile as tile
from concourse.bass_utils import run_bass_kernel_spmd

N_CORES = 8
B, C, H, W = 32, 256, 64, 64
S = H * W            # 4096 spatial positions
NB = B // N_CORES    # 4 batches per core
NCH = C // 128       # 2 channel halves of 128 partitions
NSLAB = NB * NCH     # 8 [128, S] data slabs per core

# TT emission order, and which slabs' reductions run on ACT (True) vs
# DVE (False; emitted after all TTs since DVE is in-order, so the last
# two reductions run on ACT and DVE in parallel).
SLAB_ORDER = (0, 1, 2, 3, 4, 5, 6, 7)
ACT_REDUCE = (True, True, True, True, True, True, True, False)

_CACHE = {}
LAST_RESULTS = None  # BassKernelResults of the most recent run (for test.py)


def _build_nc():
    nc = bacc.Bacc("TRN2", target_bir_lowering=False, debug=False,
                   num_devices=N_CORES)
    f16 = mybir.dt.float16
    f32 = mybir.dt.float32
    i16 = mybir.dt.int16
    # 4 "pair" tensors, one per batch, its two channel-half slabs side by
    # side.  Each pair is fetched as two full-128-partition half-DMAs (one
    # per HW queue; partition-split DMAs crater queue throughput) so batch
    # b's data completes right as its A broadcast does.
    x = nc.dram_tensor("x", [NB, 128, NCH * S], f16, kind="ExternalInput")
    aw = nc.dram_tensor("aw", [1, NB * S], f16, kind="ExternalInput")
    y = nc.dram_tensor("y", [128, NSLAB], f32, kind="ExternalOutput")
    xt, at, yt = x.ap(), aw.ap(), y.ap()
    mult = mybir.AluOpType.mult
    add = mybir.AluOpType.add

    with tile.TileContext(nc) as tc:
        with (
            tc.tile_pool(name="arow", bufs=1) as arowp,
            tc.tile_pool(name="abc", bufs=1) as abcp,
            tc.tile_pool(name="data", bufs=1) as datap,
            tc.tile_pool(name="junk", bufs=1) as junkp,
            tc.tile_pool(name="col", bufs=1) as colp,
        ):
            # A replicated across partitions by DRAM->SBUF DMAs with a
            # stride-0 source AP (rows re-read per partition; row-buffer
            # friendly), on gpsimd's software DGE queue -- keeps the two HW
            # queues free for data and gpsimd's ALU idle (its compute ops
            # share SBUF ports with DVE and throttle it 3.5x).  Two batches
            # per DMA: the A rows are contiguous in DRAM, so each transfer
            # gets 16KB packets (~140GB/s vs ~110 at 8KB) while batch pairs
            # still arrive staggered.
            abcp2 = []
            for p in range(NB // 2):
                t = abcp.tile([128, 2 * S], f16, name=f"abc{p}")
                nc.gpsimd.dma_start(
                    t[:],
                    at[0:1, 2 * p * S : 2 * (p + 1) * S]
                    .broadcast_to([128, 2 * S]),
                )
                abcp2.append(t)
            abc = [abcp2[b // 2][:, (b % 2) * S : (b % 2 + 1) * S]
                   for b in range(NB)]

            dp = []
            for j in range(NB):
                d = datap.tile([128, NCH * S], f16, name=f"dp{j}")
                nc.sync.dma_start(d[:, 0:S], xt[j][:, 0:S])
                nc.scalar.dma_start(d[:, S : 2 * S], xt[j][:, S : 2 * S])
                dp.append(d)

            prods = [junkp.tile([128, S], f16, name=f"prod{j}")
                     for j in range(4)]
            ajunk = junkp.tile([128, S], f16)  # ACT's mandatory full out
            segs = colp.tile([128, 64], f32)
            cols = colp.tile([128, NSLAB], f32)
            # Emission in data-ARRIVAL order (batch 3 rides the early-
            # finishing ring 0, so its slabs land before batch 2's).  The
            # last-arriving slab's reduce runs on DVE after all TTs so the
            # final two reductions run on ACT and DVE in parallel.
            dve_red = []
            for k, i in enumerate(SLAB_ORDER):
                b, l = i // NCH, i % NCH
                prod = prods[k % 4]
                nc.vector.tensor_tensor(
                    out=prod[:], in0=dp[b][:, l * S : (l + 1) * S],
                    in1=abc[b], op=mult)
                if ACT_REDUCE[i]:
                    nc.scalar.activation(
                        ajunk[:], prod[:],
                        mybir.ActivationFunctionType.Copy,
                        accum_out=cols[:, i : i + 1])
                else:
                    dve_red.append((i, prod))
            for i, prod in dve_red:
                nc.vector.tensor_reduce(
                    cols[:, i : i + 1], prod[:],
                    axis=mybir.AxisListType.X, op=add)
            nc.sync.dma_start(yt[:, :], cols[:])

    nc.compile()
    return nc


def _weight_field(offset, trans_std):
    """A[b,s]: accumulated bilinear weights per source pixel (UNnormalized:
    the 1/(H*W) mean factor is applied on host after the device reduction).

    Mirrors the reference coordinate math in float32.
    """
    offset = np.asarray(offset, np.float32)
    ts = np.float32(min(max(float(trans_std), 0.001), 0.01))
    ii = np.arange(H, dtype=np.float32)[None, :, None]
    jj = np.arange(W, dtype=np.float32)[None, None, :]
    y = np.clip(ii + ts * offset[:, 0] * np.float32(H),
                np.float32(0.0), np.float32(H - 1))
    x = np.clip(jj + ts * offset[:, 1] * np.float32(W),
                np.float32(0.0), np.float32(W - 1))
    y0 = np.clip(np.floor(y).astype(np.int32), 0, H - 2)
    x0 = np.clip(np.floor(x).astype(np.int32), 0, W - 2)
    wy = (y - y0.astype(np.float32)).astype(np.float64)
    wx = (x - x0.astype(np.float32)).astype(np.float64)

    base = np.arange(offset.shape[0], dtype=np.int64)[:, None, None] * S
    i00 = (y0.astype(np.int64) * W + x0 + base).ravel()
    i01 = i00 + 1
    i10 = i00 + W
    i11 = i10 + 1
    n = offset.shape[0] * S
    acc = (
        np.bincount(i00, ((1 - wy) * (1 - wx)).ravel(), minlength=n)
        + np.bincount(i01, ((1 - wy) * wx).ravel(), minlength=n)
        + np.bincount(i10, (wy * (1 - wx)).ravel(), minlength=n)
        + np.bincount(i11, (wy * wx).ravel(), minlength=n)
    )
    return acc.astype(np.float32).reshape(offset.shape[0], S)


def kernel(data, offset, trans_std):
    global LAST_RESULTS
    data = np.asarray(data, np.float32)
    offset = np.asarray(offset, np.float32)
    ts = float(np.asarray(trans_std).reshape(()))

    aw = _weight_field(offset, ts)  # [B, S] f32, unnormalized

    if "nc" not in _CACHE:
        _CACHE["nc"] = _build_nc()
    nc = _CACHE["nc"]

    # x shard layout: NB pairs of [128, 2*S] fp16; pair j = batch j's two
    # channel-half slabs side by side.
    xs = data.reshape(B, NCH, 128, S)
    in_maps = []
    for i in range(N_CORES):
        xi = (xs[i * NB : (i + 1) * NB].transpose(0, 2, 1, 3)
              .reshape(NB, 128, NCH * S).astype(np.float16))
        ai = aw[i * NB : (i + 1) * NB].reshape(1, NB * S).astype(np.float16)
        in_maps.append({"x": np.ascontiguousarray(xi),
                        "aw": np.ascontiguousarray(ai)})

    res = run_bass_kernel_spmd(nc, in_maps, core_ids=list(range(N_CORES)))
    LAST_RESULTS = res
    # y[p, b*NCH+ch] -> out[b, ch*128+p];  1/S == 2^-12 is exact in fp32.
    out = np.concatenate(
        [
            res.results[i]["y"].reshape(128, NB, NCH)
            .transpose(1, 2, 0).reshape(NB, C)
            for i in range(N_CORES)
        ],
        axis=0,
    )
    return np.ascontiguousarray((out * np.float32(1.0 / S)).astype(np.float32))
